# revision 4
# baseline (speedup 1.0000x reference)
"""Trainium2 Bass kernel for batched tanh-attention flat-softmax.

Per batch b:
    Q = query[b] @ W_query; K = query[b] @ W_key      # [S, 64]
    s = tanh(Q @ K.T) * 10                            # [S, S]
    s[diag] = -inf
    out[b] = softmax(s.flatten())

Sharding: data-parallel over batch across 8 NeuronCores (6 batches/core),
W_query/W_key replicated; no cross-core communication.

Numerics: tanh(x)*10 is bounded in [-10,10], so softmax needs no max
subtraction: out = exp(10*tanh(s)) / sum(...). The diagonal gets -30000
accumulated INTO the PSUM scores by a tiny extra matmul (identity
stationary x (-30000*I) moving), so tanh saturates to -1 and exp gives
e^-10 ~ 4.5e-5 (vs the reference's exact 0); the L2 impact is ~1e-10.
This keeps the diag handling entirely on the PE - nothing sits between
the score matmuls and tanh, so the Scalar engine never stalls.

Precision strategy (validated vs fp64 reference: rel L2 ~ 1.2e-3):
  - query cast to a single fp16 during the DMA load (SWDGE cast, free)
  - queryT built by TensorE transposes (8x [128,128] fp16; no DRAM
    round trip: saves 6MB of HBM traffic vs a DMA-transpose approach)
  - W_query|W_key stacked, single fp16
  - proj: [Q;K].T = W.T @ qT, one fp16 matmul per 512-col window
  - Q split fp16 hi/lo from fp32 PSUM; scores = [Qh;Ql].T @ [Kh;Kh]
    (one 128-contraction matmul per window; Q@Kl term dropped, ~2^-12)

The Scalar engine is the roofline (~15.35us/batch: 8x tanh[128,1024] +
1x exp[128,8192]); everything else is scheduled to keep it 100% busy:
  - sc PSUM tiles share one 3-deep ring with the proj output, so the PE
    prefills up to 3 score chunks during each exp
  - next-batch transposes are emitted after score chunk 2, landing in
    the PE's idle window during exp
  - the reciprocal for batch b's softmax sum is emitted one iteration
    later, so it never blocks next-batch operand prep in the DVE queue
  - loads are cast-DMAs issued 2 batches ahead on the gpsimd queue
  - batch 0's tanh is window-split (16x [128,512]) to start the ACT
    stream as early as possible
"""

import numpy as np

import concourse.bass as bass
import concourse.bass_isa as bass_isa
import concourse.mybir as mybir
import concourse.tile as tile
from concourse import bacc
from concourse.bass_utils import run_bass_kernel_spmd

B = 48
S = 1024
D = 128
DK = 64
N_CORES = 8
BPC = B // N_CORES
P = 128
NQ = S // P
F32 = mybir.dt.float32
F16 = mybir.dt.float16
AL = mybir.AluOpType

TANH_CLIP = 10.0
DIAG_NEG = -30000.0


def build_bass() -> bass.Bass:
    nc = bacc.Bacc(None, target_bir_lowering=False)

    q_d = nc.dram_tensor("query", [BPC, S, D], F32, kind="ExternalInput")
    wq_d = nc.dram_tensor("W_query", [D, DK], F32, kind="ExternalInput")
    wk_d = nc.dram_tensor("W_key", [D, DK], F32, kind="ExternalInput")
    out_d = nc.dram_tensor("out", [BPC, S, S], F32, kind="ExternalOutput")

    with tile.TileContext(nc) as tc:
        with (
            tc.tile_pool(name="singles", bufs=1) as singles,
            tc.tile_pool(name="qload", bufs=3) as qload,
            tc.tile_pool(name="qtp", bufs=2) as qtp,
            tc.tile_pool(name="projsb", bufs=2) as projsb,
            tc.tile_pool(name="tbuf", bufs=2) as tbuf,
            tc.tile_pool(name="obuf", bufs=3) as obuf,
            tc.tile_pool(name="small", bufs=2) as small,
            tc.tile_pool(name="ps_qt", bufs=1, space="PSUM") as ps_qt,
            tc.tile_pool(name="ps_sc", bufs=3, space="PSUM") as ps_sc,
        ):
            def load(b):
                """Cast-DMA query[b] fp32->fp16 into SBUF, in two halves
                so the first transposes can start after half the data."""
                q16 = qload.tile([P, NQ, D], F16, tag="q16", name="q16")
                hv = q_d[b].rearrange("(h n p) d -> h p n d", h=2, p=P)
                for h in range(2):
                    nc.gpsimd.dma_start(q16[:, 4 * h:4 * h + 4], hv[h])
                return q16

            # batch-0 load goes first so the DMA overlaps all the setup
            q16 = load(0)

            # --- one-time setup ---
            # fp16 identity (TensorE transpose + diag-accumulate stationary)
            ident32 = singles.tile([P, P], F32)
            nc.vector.memset(ident32, 0.0)
            nc.gpsimd.affine_select(
                out=ident32,
                in_=ident32,
                compare_op=AL.not_equal,
                fill=1.0,
                base=0,
                pattern=[[-1, P]],
                channel_multiplier=1,
            )
            ident = singles.tile([P, P], F16)
            nc.vector.tensor_copy(ident, ident32)
            # -30000 * I, the moving operand of the diag-accumulate matmul
            negd32 = singles.tile([P, P], F32)
            nc.vector.memset(negd32, 0.0)
            nc.gpsimd.affine_select(
                out=negd32,
                in_=negd32,
                compare_op=AL.not_equal,
                fill=DIAG_NEG,
                base=0,
                pattern=[[-1, P]],
                channel_multiplier=1,
            )
            negd = singles.tile([P, P], F16)
            nc.vector.tensor_copy(negd, negd32)

            # warm the ACT table set (exp_and_others holds tanh+exp) while
            # the first query load is still in flight
            actwarm = singles.tile([P, 1], F32)
            nc.scalar.activation(
                out=actwarm, in_=ident32[:, 0:1],
                func=mybir.ActivationFunctionType.Tanh,
            )

            # W stacked [Wq | Wk] as fp32, cast to a single fp16
            w32 = singles.tile([D, 2 * DK], F32)
            nc.sync.dma_start(w32[:, 0:DK], wq_d[:, :])
            nc.sync.dma_start(w32[:, DK:2 * DK], wk_d[:, :])
            w16 = singles.tile([D, 2 * DK], F16)
            nc.vector.tensor_copy(w16, w32)

            # ---- per-batch stages (window-split) ----------------------
            def prep_transpose(nq16):
                """qT[d, 128n+p] = q16[p, n, d] via TensorE transposes."""
                qT = qtp.tile([P, S], F16, tag="qT", name="qT")
                for h in range(2):
                    qtps = ps_qt.tile([P, 512], F16, tag="qt", name="qtps")
                    for i in range(4):
                        n = 4 * h + i
                        nc.tensor.transpose(
                            qtps[:, i * P:(i + 1) * P], nq16[:, n, :], ident
                        )
                    cols = slice(h * 512, (h + 1) * 512)
                    nc.vector.tensor_copy(qT[:, cols], qtps)
                return qT

            def prep_stacks(qT):
                """proj + fp16 hi/lo split + stacked matmul operands."""
                pp = ps_sc.tile([P, S], F32, tag="sc", name="pp")
                hb = projsb.tile([P, S], F16, tag="hb")    # [Qh; Kh]
                lb = projsb.tile([DK, S], F16, tag="lb")   # Ql
                qstack = projsb.tile([P, S], F16, tag="qstack")  # [Qh; Ql]
                khh = projsb.tile([P, S], F16, tag="khh")        # [Kh; Kh]
                for h in range(2):
                    cols = slice(h * 512, (h + 1) * 512)
                    nc.tensor.matmul(
                        pp[:, cols], w16, qT[:, cols], start=True, stop=True
                    )
                    nc.vector.tensor_copy(hb[:, cols], pp[:, cols])
                    nc.vector.tensor_tensor(
                        lb[:, cols], pp[0:DK, cols], hb[0:DK, cols],
                        AL.subtract,
                    )
                    nc.vector.tensor_copy(qstack[0:DK, cols], hb[0:DK, cols])
                    nc.vector.tensor_copy(qstack[DK:P, cols], lb[:, cols])
                    nc.vector.tensor_copy(khh[0:DK, cols], hb[DK:P, cols])
                    nc.vector.tensor_copy(khh[DK:P, cols], hb[DK:P, cols])
                return qstack, khh

            def score_chunk(t16, qstack, khh, c, split_tanh):
                """One 128-row score chunk: 2 matmuls + diag accum + tanh."""
                sc = ps_sc.tile([P, S], F32, tag="sc", name="sc")
                hd = c // 4  # window containing this chunk's diag block
                for h in range(2):
                    cols = slice(h * 512, (h + 1) * 512)
                    nc.tensor.matmul(
                        sc[:, cols], qstack[:, c * P:(c + 1) * P],
                        khh[:, cols], start=True, stop=(h != hd),
                    )
                    if h == hd:
                        nc.tensor.matmul(
                            sc[:, c * P:(c + 1) * P], ident, negd,
                            start=False, stop=True, skip_group_check=True,
                        )
                    if split_tanh:
                        nc.scalar.activation(
                            out=t16[:, c, cols], in_=sc[:, cols],
                            func=mybir.ActivationFunctionType.Tanh,
                        )
                if not split_tanh:
                    nc.scalar.activation(
                        out=t16[:, c], in_=sc,
                        func=mybir.ActivationFunctionType.Tanh,
                    )

            def exp_batch(t16, o32, rs):
                nc.scalar.activation(
                    out=o32, in_=t16,
                    func=mybir.ActivationFunctionType.Exp,
                    scale=TANH_CLIP,
                    accum_out=rs,
                )
                zall = small.tile([P, 1], F32, tag="zall")
                nc.gpsimd.partition_all_reduce(
                    zall, rs, channels=P, reduce_op=bass_isa.ReduceOp.add
                )
                return zall

            def norm_store(b, o32, zall, ngrp):
                """reciprocal (deferred to this iteration so it never
                blocks prep work in the DVE queue) + normalize + store."""
                rz = small.tile([P, 1], F32, tag="rz")
                nc.vector.reciprocal(rz, zall)
                ov = out_d[b].rearrange("(n p) s -> p n s", p=P)
                w = NQ // ngrp
                for g in range(ngrp):
                    sl = slice(w * g, w * (g + 1))
                    nc.vector.tensor_scalar_mul(o32[:, sl], o32[:, sl], rz)
                    nc.sync.dma_start(ov[:, sl], o32[:, sl])

            # ---- software-pipelined batch loop ------------------------
            qT = prep_transpose(q16)
            if BPC > 1:
                nq16 = load(1)
            ops = prep_stacks(qT)
            pending = None  # (b, o32, zall) awaiting recip+normalize+store

            for b in range(BPC):
                t16 = tbuf.tile([P, NQ, S], F16, tag="t16")
                o32 = obuf.tile([P, NQ, S], F32, tag="o32")
                rs = small.tile([P, 1], F32, tag="rs")

                if b + 2 < BPC:
                    nnq16 = load(b + 2)

                for c in range(3):
                    score_chunk(t16, *ops, c, b == 0)
                if b + 1 < BPC:
                    # transposes land in the PE's idle window during exp_b
                    nqT = prep_transpose(nq16)
                    nq16 = nnq16 if b + 2 < BPC else None
                for c in range(3, NQ):
                    score_chunk(t16, *ops, c, b == 0)
                if b + 1 < BPC:
                    ops = prep_stacks(nqT)

                if pending is not None:
                    norm_store(*pending, ngrp=4)
                    pending = None

                zall = exp_batch(t16, o32, rs)
                pending = (b, o32, zall)

            # fine-grained tail: the last batch's store is the critical path
            norm_store(*pending, ngrp=8)

    nc.compile()
    return nc


_CACHED_NC = None


def kernel(**inputs: np.ndarray) -> np.ndarray:
    global _CACHED_NC
    query = np.ascontiguousarray(np.asarray(inputs["query"], dtype=np.float32))
    wq = np.ascontiguousarray(np.asarray(inputs["W_query"], dtype=np.float32))
    wk = np.ascontiguousarray(np.asarray(inputs["W_key"], dtype=np.float32))
    assert query.shape == (B, S, D), query.shape

    if _CACHED_NC is None:
        _CACHED_NC = build_bass()
    nc = _CACHED_NC

    in_maps = [
        {
            "query": query[c * BPC:(c + 1) * BPC],
            "W_query": wq,
            "W_key": wk,
        }
        for c in range(N_CORES)
    ]
    res = run_bass_kernel_spmd(nc, in_maps, core_ids=list(range(N_CORES)))
    out = np.concatenate(
        [r["out"].reshape(BPC, S * S) for r in res.results], axis=0
    )
    return out


# revision 5
# speedup vs baseline: 1.0042x; 1.0042x over previous
"""Trainium2 Bass kernel for batched tanh-attention flat-softmax.

Per batch b:
    Q = query[b] @ W_query; K = query[b] @ W_key      # [S, 64]
    s = tanh(Q @ K.T) * 10                            # [S, S]
    s[diag] = -inf
    out[b] = softmax(s.flatten())

Sharding: data-parallel over batch across 8 NeuronCores (6 batches/core),
W_query/W_key replicated; no cross-core communication.

Numerics: tanh(x)*10 is bounded in [-10,10], so softmax needs no max
subtraction: out = exp(10*tanh(s)) / sum(...). The diagonal gets -30000
accumulated INTO the PSUM scores by a tiny extra matmul (identity
stationary x (-30000*I) moving), so tanh saturates to -1 and exp gives
e^-10 ~ 4.5e-5 (vs the reference's exact 0); the L2 impact is ~1e-10.
This keeps the diag handling entirely on the PE - nothing sits between
the score matmuls and tanh, so the Scalar engine never stalls.

Precision strategy (validated vs fp64 reference: rel L2 ~ 1.2e-3):
  - query cast to a single fp16 during the DMA load (SWDGE cast, free)
  - queryT built by TensorE transposes (8x [128,128] fp16; no DRAM
    round trip: saves 6MB of HBM traffic vs a DMA-transpose approach)
  - W_query|W_key stacked, single fp16
  - proj: [Q;K].T = W.T @ qT, one fp16 matmul per 512-col window
  - Q split fp16 hi/lo from fp32 PSUM; scores = [Qh;Ql].T @ [Kh;Kh]
    (one 128-contraction matmul per window; Q@Kl term dropped, ~2^-12)

The Scalar engine is the roofline (~15.35us/batch: 8x tanh[128,1024] +
1x exp[128,8192]); everything else is scheduled to keep it 100% busy:
  - sc PSUM tiles share one 3-deep ring with the proj output, so the PE
    prefills up to 3 score chunks during each exp
  - next-batch transposes are emitted after score chunk 2, landing in
    the PE's idle window during exp
  - the reciprocal for batch b's softmax sum is emitted one iteration
    later, so it never blocks next-batch operand prep in the DVE queue
  - loads are cast-DMAs issued 2 batches ahead on the gpsimd queue
  - batch 0's tanh is window-split (16x [128,512]) to start the ACT
    stream as early as possible
"""

import numpy as np

import concourse.bass as bass
import concourse.bass_isa as bass_isa
import concourse.mybir as mybir
import concourse.tile as tile
from concourse import bacc
from concourse.bass_utils import run_bass_kernel_spmd

B = 48
S = 1024
D = 128
DK = 64
N_CORES = 8
BPC = B // N_CORES
P = 128
NQ = S // P
F32 = mybir.dt.float32
F16 = mybir.dt.float16
AL = mybir.AluOpType

TANH_CLIP = 10.0
DIAG_NEG = -30000.0


def build_bass() -> bass.Bass:
    nc = bacc.Bacc(None, target_bir_lowering=False)

    q_d = nc.dram_tensor("query", [BPC, S, D], F32, kind="ExternalInput")
    wq_d = nc.dram_tensor("W_query", [D, DK], F32, kind="ExternalInput")
    wk_d = nc.dram_tensor("W_key", [D, DK], F32, kind="ExternalInput")
    out_d = nc.dram_tensor("out", [BPC, S, S], F32, kind="ExternalOutput")

    with tile.TileContext(nc) as tc:
        with (
            tc.tile_pool(name="singles", bufs=1) as singles,
            tc.tile_pool(name="qload", bufs=3) as qload,
            tc.tile_pool(name="qtp", bufs=2) as qtp,
            tc.tile_pool(name="projsb", bufs=2) as projsb,
            tc.tile_pool(name="tbuf", bufs=2) as tbuf,
            tc.tile_pool(name="obuf", bufs=3) as obuf,
            tc.tile_pool(name="small", bufs=2) as small,
            tc.tile_pool(name="ps_qt", bufs=1, space="PSUM") as ps_qt,
            tc.tile_pool(name="ps_sc", bufs=3, space="PSUM") as ps_sc,
        ):
            def load(b):
                """Cast-DMA query[b] fp32->fp16 into SBUF, in two halves
                so the first transposes can start after half the data."""
                q16 = qload.tile([P, NQ, D], F16, tag="q16", name="q16")
                hv = q_d[b].rearrange("(h n p) d -> h p n d", h=2, p=P)
                for h in range(2):
                    nc.gpsimd.dma_start(q16[:, 4 * h:4 * h + 4], hv[h])
                return q16

            # batch-0 load goes first so the DMA overlaps all the setup
            q16 = load(0)

            # --- one-time setup ---
            # fp16 identity (TensorE transpose + diag-accumulate stationary)
            ident32 = singles.tile([P, P], F32)
            nc.vector.memset(ident32, 0.0)
            nc.gpsimd.affine_select(
                out=ident32,
                in_=ident32,
                compare_op=AL.not_equal,
                fill=1.0,
                base=0,
                pattern=[[-1, P]],
                channel_multiplier=1,
            )
            ident = singles.tile([P, P], F16)
            nc.vector.tensor_copy(ident, ident32)
            # -30000 * I, the moving operand of the diag-accumulate matmul
            negd32 = singles.tile([P, P], F32)
            nc.vector.memset(negd32, 0.0)
            nc.gpsimd.affine_select(
                out=negd32,
                in_=negd32,
                compare_op=AL.not_equal,
                fill=DIAG_NEG,
                base=0,
                pattern=[[-1, P]],
                channel_multiplier=1,
            )
            negd = singles.tile([P, P], F16)
            nc.vector.tensor_copy(negd, negd32)

            # warm the ACT table set (exp_and_others holds tanh+exp) while
            # the first query load is still in flight
            actwarm = singles.tile([P, 1], F32)
            nc.scalar.activation(
                out=actwarm, in_=ident32[:, 0:1],
                func=mybir.ActivationFunctionType.Tanh,
            )

            # W stacked [Wq | Wk] as fp32, cast to a single fp16
            w32 = singles.tile([D, 2 * DK], F32)
            nc.sync.dma_start(w32[:, 0:DK], wq_d[:, :])
            nc.sync.dma_start(w32[:, DK:2 * DK], wk_d[:, :])
            w16 = singles.tile([D, 2 * DK], F16)
            nc.vector.tensor_copy(w16, w32)

            # ---- per-batch stages (window-split) ----------------------
            def prep_transpose(nq16):
                """qT[d, 128n+p] = q16[p, n, d] via TensorE transposes."""
                qT = qtp.tile([P, S], F16, tag="qT", name="qT")
                for h in range(2):
                    qtps = ps_qt.tile([P, 512], F16, tag="qt", name="qtps")
                    for i in range(4):
                        n = 4 * h + i
                        nc.tensor.transpose(
                            qtps[:, i * P:(i + 1) * P], nq16[:, n, :], ident
                        )
                    cols = slice(h * 512, (h + 1) * 512)
                    nc.vector.tensor_copy(qT[:, cols], qtps)
                return qT

            def prep_stacks(qT):
                """proj + fp16 hi/lo split + stacked matmul operands."""
                pp = ps_sc.tile([P, S], F32, tag="sc", name="pp")
                hb = projsb.tile([P, S], F16, tag="hb")    # [Qh; Kh]
                lb = projsb.tile([DK, S], F16, tag="lb")   # Ql
                qstack = projsb.tile([P, S], F16, tag="qstack")  # [Qh; Ql]
                khh = projsb.tile([P, S], F16, tag="khh")        # [Kh; Kh]
                for h in range(2):
                    cols = slice(h * 512, (h + 1) * 512)
                    nc.tensor.matmul(
                        pp[:, cols], w16, qT[:, cols], start=True, stop=True
                    )
                    nc.vector.tensor_copy(hb[:, cols], pp[:, cols])
                    nc.vector.tensor_tensor(
                        lb[:, cols], pp[0:DK, cols], hb[0:DK, cols],
                        AL.subtract,
                    )
                    nc.vector.tensor_copy(qstack[0:DK, cols], hb[0:DK, cols])
                    nc.vector.tensor_copy(qstack[DK:P, cols], lb[:, cols])
                    nc.vector.tensor_copy(khh[0:DK, cols], hb[DK:P, cols])
                    nc.vector.tensor_copy(khh[DK:P, cols], hb[DK:P, cols])
                return qstack, khh

            def score_chunk(t16, qstack, khh, c):
                """One 128-row score chunk: 2 matmuls + diag accum + tanh."""
                sc = ps_sc.tile([P, S], F32, tag="sc", name="sc")
                hd = c // 4  # window containing this chunk's diag block
                for h in range(2):
                    cols = slice(h * 512, (h + 1) * 512)
                    nc.tensor.matmul(
                        sc[:, cols], qstack[:, c * P:(c + 1) * P],
                        khh[:, cols], start=True, stop=(h != hd),
                    )
                    if h == hd:
                        nc.tensor.matmul(
                            sc[:, c * P:(c + 1) * P], ident, negd,
                            start=False, stop=True, skip_group_check=True,
                        )
                if True:
                    nc.scalar.activation(
                        out=t16[:, c], in_=sc,
                        func=mybir.ActivationFunctionType.Tanh,
                    )

            def exp_batch(t16, o32, rs):
                nc.scalar.activation(
                    out=o32, in_=t16,
                    func=mybir.ActivationFunctionType.Exp,
                    scale=TANH_CLIP,
                    accum_out=rs,
                )
                zall = small.tile([P, 1], F32, tag="zall")
                nc.gpsimd.partition_all_reduce(
                    zall, rs, channels=P, reduce_op=bass_isa.ReduceOp.add
                )
                return zall

            def norm_store(b, o32, zall, ngrp):
                """reciprocal (deferred to this iteration so it never
                blocks prep work in the DVE queue) + normalize + store."""
                rz = small.tile([P, 1], F32, tag="rz")
                nc.vector.reciprocal(rz, zall)
                ov = out_d[b].rearrange("(n p) s -> p n s", p=P)
                w = NQ // ngrp
                for g in range(ngrp):
                    sl = slice(w * g, w * (g + 1))
                    nc.vector.tensor_scalar_mul(o32[:, sl], o32[:, sl], rz)
                    nc.sync.dma_start(ov[:, sl], o32[:, sl])

            # ---- software-pipelined batch loop ------------------------
            qT = prep_transpose(q16)
            if BPC > 1:
                nq16 = load(1)
            ops = prep_stacks(qT)
            pending = None  # (b, o32, zall) awaiting recip+normalize+store

            for b in range(BPC):
                t16 = tbuf.tile([P, NQ, S], F16, tag="t16")
                o32 = obuf.tile([P, NQ, S], F32, tag="o32")
                rs = small.tile([P, 1], F32, tag="rs")

                if b + 2 < BPC:
                    nnq16 = load(b + 2)

                for c in range(3):
                    score_chunk(t16, *ops, c)
                if b + 1 < BPC:
                    # transposes land in the PE's idle window during exp_b
                    nqT = prep_transpose(nq16)
                    nq16 = nnq16 if b + 2 < BPC else None
                for c in range(3, NQ):
                    score_chunk(t16, *ops, c)
                if b + 1 < BPC:
                    ops = prep_stacks(nqT)

                if pending is not None:
                    norm_store(*pending, ngrp=4)
                    pending = None

                zall = exp_batch(t16, o32, rs)
                pending = (b, o32, zall)

            # fine-grained tail: the last batch's store is the critical path
            norm_store(*pending, ngrp=8)

    nc.compile()
    return nc


_CACHED_NC = None


def kernel(**inputs: np.ndarray) -> np.ndarray:
    global _CACHED_NC
    query = np.ascontiguousarray(np.asarray(inputs["query"], dtype=np.float32))
    wq = np.ascontiguousarray(np.asarray(inputs["W_query"], dtype=np.float32))
    wk = np.ascontiguousarray(np.asarray(inputs["W_key"], dtype=np.float32))
    assert query.shape == (B, S, D), query.shape

    if _CACHED_NC is None:
        _CACHED_NC = build_bass()
    nc = _CACHED_NC

    in_maps = [
        {
            "query": query[c * BPC:(c + 1) * BPC],
            "W_query": wq,
            "W_key": wk,
        }
        for c in range(N_CORES)
    ]
    res = run_bass_kernel_spmd(nc, in_maps, core_ids=list(range(N_CORES)))
    out = np.concatenate(
        [r["out"].reshape(BPC, S * S) for r in res.results], axis=0
    )
    return out
